# revision 22
# baseline (speedup 1.0000x reference)
"""CapsNet (semantic capsules + dynamic routing) on 8 TRN2 NeuronCores.

Sharding: stage A (fc1+squash) is sequence-sharded (core i owns 32 of 256
positions), then ONE small AllGather of u (393KB/core) gives every core the
full routing input.  Stages B-D are capsule-sharded: core c loads only
capsule c's route_weights (12.6MB, read exactly once machine-wide),
computes capsule c's full priors locally (no ReduceScatter, no partial-
priors DRAM round-trip), runs dynamic routing for capsule c, and emits
output batches 8c..8c+8 (the reference's flat reinterpret of vote maps
capsule c exactly onto those batches).

The stage-D reinterpret (vote[b, sl*8+c'] -> h[row, c']) is done with 32
on-chip PE transposes into an (sl, m, j)-ordered hT; the permutation back
to (m, j, sl) row order is folded into the final output DMA (1.5KB runs).
Final matmul + output store in bf16 (validated ~4e-3 rel err, budget 2e-2).
"""
import sys
from contextlib import ExitStack

if '/opt/trn_rl_repo' not in sys.path:
    sys.path.insert(0, '/opt/trn_rl_repo')

import numpy as np
import ml_dtypes

import concourse.bass as bass
import concourse.bacc as bacc
import concourse.tile as tile
from concourse import mybir
import concourse.bass_utils as bass_utils

F32 = mybir.dt.float32
BF16 = mybir.dt.bfloat16
AX = mybir.AxisListType
ALU = mybir.AluOpType
ACTF = mybir.ActivationFunctionType

N_CORES = 8
B, S, D = 64, 256, 768
CAP, NT = 8, 10
NCOL = NT * CAP          # 80 fc1 output cols (n*8+c)
SL = S // N_CORES        # 32 sequence positions per core (stage A)
KL = SL * CAP            # 256 local contraction elements
K = S * CAP              # 2048 full contraction
KT = K // 128            # 16 k-tiles
L = S                    # 256 class dim
BLOC = B // N_CORES      # 8 output batches per core

_cache = {}


def _build(R: int, debug_mode: int = 0):
    """Build + compile the SPMD program for R active routes.

    debug_mode=1: f32 output with taps of every stage's intermediates.
    """
    nc = bacc.Bacc("TRN2", target_bir_lowering=False, debug=False,
                   num_devices=N_CORES)

    xt = nc.dram_tensor("xt", [D, SL * B], F32, kind="ExternalInput")
    fw = nc.dram_tensor("fw", [128, 6 * NCOL], F32, kind="ExternalInput")
    fb = nc.dram_tensor("fb", [NCOL, 1], F32, kind="ExternalInput")
    rw = nc.dram_tensor("rw", [KT, 128, R * L], F32, kind="ExternalInput")
    lwt = nc.dram_tensor("lwt", [CAP, D], BF16, kind="ExternalInput")
    ident = nc.dram_tensor("ident", [128, 128], F32, kind="ExternalInput")
    out = nc.dram_tensor("out", [BLOC * S, D], F32 if debug_mode else BF16,
                         kind="ExternalOutput")

    RB = R * B               # 384: (r, b) block per k-tile
    uTd = nc.dram_tensor("uTd", [2, 128, RB], F32)
    uTg = [nc.dram_tensor(f"uTg{h}", [N_CORES, 128, RB], F32,
                          addr_space="Shared") for h in range(2)]
    # stage-B consumption order: even k-tiles (h=0) first, then odd;
    # within each h, tiles 2.. first so the last-created rw bufs recycle early
    CHAIN = ([2 * g for g in range(2, N_CORES)] + [0, 2] +
             [2 * g + 1 for g in range(2, N_CORES)] + [1, 3])

    NPAIR = (R + 1) // 2

    ecnt = [0]

    def copy_alt(dst, src):
        """Alternate PSUM->SBUF copies between ACT and DVE."""
        ecnt[0] += 1
        if ecnt[0] % 2 == 0:
            nc.scalar.copy(dst, src)
        else:
            nc.vector.tensor_copy(dst, src)

    with tile.TileContext(nc) as tc:
        with (
            tc.tile_pool(name="const", bufs=1) as constp,
            tc.tile_pool(name="rwp", bufs=12) as rwp,
            tc.tile_pool(name="uTall", bufs=1) as uTallp,
            tc.tile_pool(name="junk", bufs=2) as junkp,
            tc.tile_pool(name="route", bufs=1) as rt,
            tc.tile_pool(name="acc", bufs=2) as accp,
            tc.tile_pool(name="osb", bufs=4) as osbp,
        ):
            # ---- constants in ----
            fw_sb = constp.tile([128, 6 * NCOL], F32, tag="fw")
            nc.sync.dma_start(out=fw_sb[:], in_=fw[:])
            fb_sb = constp.tile([NCOL, 1], F32, tag="fb")
            nc.sync.dma_start(out=fb_sb[:], in_=fb[:])
            lwt_sb = constp.tile([CAP, D], BF16, tag="lwt")
            nc.sync.dma_start(out=lwt_sb[:], in_=lwt[:])
            id_sb = constp.tile([128, 128], F32, tag="ident")
            nc.sync.dma_start(out=id_sb[:], in_=ident[:])

            # ================= stage A: fc1 -> uT ======================
            sa_stack = ExitStack()
            sa = sa_stack.enter_context(tc.tile_pool(name="stageA", bufs=1))
            xtp = sa_stack.enter_context(tc.tile_pool(name="xtp", bufs=4))
            psA = sa_stack.enter_context(
                tc.tile_pool(name="psA", bufs=1, space="PSUM"))

            # xt first: stage A's critical input must not queue behind the
            # 12.6MB route_weights prefetch
            xt_t = []
            for j in range(6):
                t = xtp.tile([128, SL * B], F32, tag="xt")
                nc.sync.dma_start(out=t[:], in_=xt[j * 128:(j + 1) * 128, :])
                xt_t.append(t)

            # ---- PE warmup: ramp tensor engine while DMAs stream ----
            warm_stack = ExitStack()
            psW = warm_stack.enter_context(
                tc.tile_pool(name="psW", bufs=2, space="PSUM"))
            for w in range(14):
                pj = psW.tile([128, 128], F32, tag="warm", name="warm")
                nc.tensor.transpose(pj[:], id_sb[:], id_sb[:])

            # ---- route_weights prefetch (fills queues behind xt) ----
            rw_t = {}
            for t in CHAIN:
                rwt = rwp.tile([128, R * L], F32, tag="rw", name=f"rw{t}")
                nc.sync.dma_start(out=rwt[:], in_=rw[t])
                rw_t[t] = rwt

            psum_sem = psA.tile([NCOL, SL * B], F32, tag="sem")
            for j in range(6):
                for n4 in range(4):
                    nc.tensor.matmul(
                        psum_sem[:, n4 * 512:(n4 + 1) * 512],
                        lhsT=fw_sb[:, j * NCOL:(j + 1) * NCOL],
                        rhs=xt_t[j][:, n4 * 512:(n4 + 1) * 512],
                        start=(j == 0), stop=(j == 5),
                    )
            warm_stack.close()
            psT = sa_stack.enter_context(
                tc.tile_pool(name="psT", bufs=3, space="PSUM"))
            semT_sb = sa.tile([NCOL, SL * B], F32, tag="semT")
            # evacuate PSUM + add fc1 bias (per-partition scalar)
            nc.vector.tensor_scalar_add(semT_sb[:], psum_sem[:], fb_sb[0:NCOL, 0:1])

            # per-s transpose: semT [80, 64] -> u_all [64(b), s*80+nc]
            u_all = sa.tile([B, SL * NCOL], F32, tag="u_all")
            for s in range(SL):
                ps_t = psT.tile([B, NCOL], F32, tag="pst")
                nc.tensor.transpose(
                    ps_t[:], semT_sb[:, s * B:(s + 1) * B], id_sb[0:NCOL, 0:NCOL])
                copy_alt(u_all[:, s * NCOL:(s + 1) * NCOL], ps_t[:])

            # squash over n (free-strided)
            tmp2 = sa.tile([B, SL * NCOL], F32, tag="tmp2")
            nc.vector.tensor_mul(tmp2[:], u_all[:], u_all[:])
            sq = sa.tile([B, SL * CAP], F32, tag="sq")
            nc.vector.tensor_reduce(
                out=sq[:].rearrange("p (s c) -> p s c", c=CAP),
                in_=tmp2[:].rearrange("p (s n c) -> p s c n", n=NT, c=CAP),
                axis=AX.X, op=ALU.add,
            )
            s1 = sa.tile([B, SL * CAP], F32, tag="s1")
            nc.scalar.activation(s1[:], sq[:], ACTF.Sqrt)
            s2 = sa.tile([B, SL * CAP], F32, tag="s2")
            nc.vector.tensor_scalar_add(s2[:], sq[:], 1.0)
            s3 = sa.tile([B, SL * CAP], F32, tag="s3")
            nc.vector.reciprocal(s3[:], s2[:])
            scl = sa.tile([B, SL * CAP], F32, tag="scl")
            nc.vector.tensor_mul(scl[:], s1[:], s3[:])
            # u_act[b, r*256 + s*8 + c] = u_all[b, s*80 + r*8 + c] * scl[b, s*8+c]
            u_act = sa.tile([B, R * KL], F32, tag="u_act")
            u_all_r = u_all[:].rearrange("p (s n c) -> p n s c", n=NT, c=CAP)
            for r in range(R):
                nc.vector.tensor_mul(
                    u_act[:, r * KL:(r + 1) * KL], u_all_r[:, r, :, :], scl[:])

            u_act2 = None
            if debug_mode == 1:
                u_act2 = rt.tile([B, R * KL], F32, tag="uact2")
                nc.vector.tensor_copy(u_act2[:], u_act[:])

            # uT_sb[p, h*RB + r*64 + b] = u_act[b, r*256 + h*128 + p]
            # h-major so AG chunk h=0 launches while h=1 transposes run
            uT_sb = sa.tile([128, 2 * RB], F32, tag="uT")
            for h in range(2):
                for r in range(R):
                    psU = psT.tile([128, B], F32, tag="pst")
                    nc.tensor.transpose(
                        psU[:],
                        u_act[:, r * KL + h * 128:r * KL + (h + 1) * 128],
                        id_sb[0:B, 0:B],
                    )
                    copy_alt(uT_sb[:, h * RB + r * B:h * RB + (r + 1) * B], psU[:])
                nc.sync.dma_start(out=uTd[h],
                                  in_=uT_sb[:, h * RB:(h + 1) * RB])
                nc.gpsimd.collective_compute(
                    "AllGather", ALU.bypass,
                    replica_groups=[list(range(N_CORES))],
                    ins=[uTd[h:h + 1]], outs=[uTg[h][:]],
                )
            sa_stack.close()

            # uT_all[p, (h*8 + g)*RB + r*64 + b], k-tile t = 2g + h
            uT_all = uTallp.tile([128, KT * RB], F32, tag="uTall")
            HRB = N_CORES * RB
            for h in range(2):
                nc.sync.dma_start(
                    out=uT_all[:, h * HRB:(h + 1) * HRB]
                        .rearrange("p (g x) -> p g x", g=N_CORES),
                    in_=uTg[h][:].rearrange("g p x -> p g x"))

            def uT_lhsT(t, r):
                base = ((t % 2) * N_CORES + t // 2) * RB + r * B
                return uT_all[:, base:base + B]

            # ================= stage B: full priors for capsule c =======
            ps_stack = ExitStack()
            # one accumulation chain per PSUM bank: matmul start=True zeroes
            # the WHOLE bank, so chains must not share one
            psB = ps_stack.enter_context(
                tc.tile_pool(name="psB", bufs=R, space="PSUM"))
            pspri = [psB.tile([B, 2 * L], F32, tag="pri", name=f"pri{r}")
                     for r in range(R)]
            for ci, t in enumerate(CHAIN):
                for r in range(R):
                    nc.tensor.matmul(
                        pspri[r][:, 0:L],
                        lhsT=uT_lhsT(t, r),
                        rhs=rw_t[t][:, r * L:(r + 1) * L],
                        start=(ci == 0), stop=(ci == KT - 1),
                    )
            # priors -> SBUF [64, R*256]
            pri = rt.tile([B, R * L], F32, tag="pri")
            for r in range(R):
                copy_alt(pri[:, r * L:(r + 1) * L], pspri[r][:, 0:L])
            ps_stack.close()

            def pri_r(r):
                return pri[:, r * L:(r + 1) * L]

            # ============= stage C: dynamic routing ================
            # iter 0: probs uniform over R active routes.
            ssum = rt.tile([B, L], F32, tag="ssum")
            if R == 1:
                nc.vector.tensor_copy(ssum[:], pri_r(0))
            else:
                nc.vector.tensor_reduce(
                    out=ssum[:], in_=pri[:].rearrange("p (r l) -> p l r", r=R),
                    axis=AX.X, op=ALU.add)

            logits = rt.tile([B, R], F32, tag="logits")
            vote = rt.tile([B, L], F32, tag="vote")

            def squash_scale(v, sqscale, tag):
                """[B,1] tile: sqrt(sq)/(1+sq), sq = sum(v*v)*sqscale."""
                sqv = rt.tile([B, 1], F32, tag=tag + "sq", name=tag + "sq")
                junk = junkp.tile([B, L], F32, tag="junk", name="junk")
                sqr = rt.tile([B, 1], F32, tag=tag + "sr", name=tag + "sr")
                nc.vector.scalar_tensor_tensor(
                    out=junk[:], in0=v, scalar=1.0, in1=v,
                    op0=ALU.mult, op1=ALU.mult, accum_out=sqr[:])
                nc.vector.tensor_scalar_mul(sqv[:], sqr[:], float(sqscale))
                a = rt.tile([B, 1], F32, tag=tag + "a", name=tag + "a")
                nc.scalar.activation(a[:], sqv[:], ACTF.Sqrt)
                bb = rt.tile([B, 1], F32, tag=tag + "b", name=tag + "b")
                nc.vector.tensor_scalar_add(bb[:], sqv[:], 1.0)
                cc = rt.tile([B, 1], F32, tag=tag + "c", name=tag + "c")
                nc.vector.reciprocal(cc[:], bb[:])
                sc = rt.tile([B, 1], F32, tag=tag + "s", name=tag + "s")
                nc.vector.tensor_mul(sc[:], a[:], cc[:])
                return sc

            def raw_delta(vsrc, dst):
                """dst[b, r] = sum_l pri_r * vsrc."""
                for r in range(R):
                    junk = junkp.tile([B, L], F32, tag="junk", name="junk")
                    nc.vector.scalar_tensor_tensor(
                        out=junk[:], in0=pri_r(r), scalar=1.0, in1=vsrc,
                        op0=ALU.mult, op1=ALU.mult,
                        accum_out=dst[:, r:r + 1])

            def softmax_and_vote(lg, vdst):
                mx = rt.tile([B, 1], F32, tag="mx", name="mx")
                nc.vector.tensor_reduce(out=mx[:], in_=lg[:], axis=AX.X,
                                        op=ALU.max)
                ngm = rt.tile([B, 1], F32, tag="ngm", name="ngm")
                nc.vector.tensor_scalar_mul(ngm[:], mx[:], -1.0)
                ex = rt.tile([B, R], F32, tag="ex", name="ex")
                nc.scalar.activation(ex[:], lg[:], ACTF.Exp,
                                     bias=ngm[0:B, 0:1])
                se = rt.tile([B, 1], F32, tag="se", name="se")
                nc.vector.tensor_reduce(out=se[:], in_=ex[:], axis=AX.X,
                                        op=ALU.add)
                ri = rt.tile([B, 1], F32, tag="ri", name="ri")
                nc.vector.reciprocal(ri[:], se[:])
                pr = rt.tile([B, R], F32, tag="pr", name="pr")
                nc.vector.tensor_scalar_mul(pr[:], ex[:], ri[0:B, 0:1])
                # vote = sum_r probs_r * pri_r
                acc = accp.tile([B, L], F32, tag="acc", name="acc")
                nc.vector.tensor_scalar_mul(acc[:], pri_r(0), pr[0:B, 0:1])
                for r in range(1, R):
                    acc2 = accp.tile([B, L], F32, tag="acc", name="acc")
                    nc.vector.scalar_tensor_tensor(
                        out=acc2[:], in0=pri_r(r), scalar=pr[0:B, r:r + 1],
                        in1=acc[:], op0=ALU.mult, op1=ALU.add)
                    acc = acc2
                nc.vector.tensor_copy(vdst, acc[:])

            # iter 0
            sc0 = squash_scale(ssum[:], 1.0 / (R * R), "i0")
            rd0 = rt.tile([B, R], F32, tag="rd0")
            raw_delta(ssum[:], rd0)
            t0 = rt.tile([B, R], F32, tag="t0")
            nc.vector.tensor_scalar_mul(t0[:], rd0[:], sc0[0:B, 0:1])
            nc.vector.tensor_scalar_mul(logits[:], t0[:], 1.0 / R)

            # iter 1
            softmax_and_vote(logits, vote[:])
            sc1 = squash_scale(vote[:], 1.0, "i1")
            rd1 = rt.tile([B, R], F32, tag="rd1")
            raw_delta(vote[:], rd1)
            t1 = rt.tile([B, R], F32, tag="t1")
            nc.vector.tensor_scalar_mul(t1[:], rd1[:], sc1[0:B, 0:1])
            lg2 = rt.tile([B, R], F32, tag="lg2")
            nc.vector.tensor_add(lg2[:], logits[:], t1[:])

            # iter 2 (final vote; reference uses the un-squashed vote)
            softmax_and_vote(lg2, vote[:])

            if debug_mode == 1:
                # taps: rows 0:64 u_act[:, 0:768]; 64:128 u_act[:, 768:1536];
                # 128:256 uT_all[:, 0:768]; 256:384 uT_all[:, 5376:6144];
                # 384:448 pri[:, 0:768]; 448:512 pri[:, 768:1536];
                # 512:576 vote; 576:640 logits2
                nc.sync.dma_start(out=out[0:64, :], in_=u_act2[:, 0:768])
                nc.sync.dma_start(out=out[64:128, :], in_=u_act2[:, 768:R * KL])
                nc.sync.dma_start(out=out[128:256, :], in_=uT_all[:, 0:768])
                nc.sync.dma_start(out=out[256:384, 0:RB],
                                  in_=uT_all[:, (KT - 1) * RB:KT * RB])
                nc.sync.dma_start(out=out[384:448, :], in_=pri[:, 0:768])
                nc.sync.dma_start(out=out[448:512, :], in_=pri[:, 768:R * L])
                nc.sync.dma_start(out=out[512:576, 0:L], in_=vote[:])
                nc.sync.dma_start(out=out[576:640, 0:R], in_=lg2[:])

            if debug_mode == 0:
                # ============= stage D: reinterpret + final matmul =====
                # hTa[c, sl*64 + (m*8+j)] = vote[m*8+j, sl*8+c]  (bf16)
                ps_stack = ExitStack()
                psS = ps_stack.enter_context(
                    tc.tile_pool(name="psS", bufs=3, space="PSUM"))
                psO = ps_stack.enter_context(
                    tc.tile_pool(name="psO", bufs=2, space="PSUM"))
                hTa = rt.tile([CAP, BLOC * S], BF16, tag="hTa")
                for sl in range(SL):
                    psV = psS.tile([CAP, B], F32, tag="psv")
                    nc.tensor.transpose(
                        psV[:], vote[:, sl * CAP:(sl + 1) * CAP], id_sb[0:B, 0:B])
                    copy_alt(hTa[:, sl * B:(sl + 1) * B], psV[:])

                for t in range(16):
                    pso = psO.tile([128, D], F32, tag="pso")
                    nc.tensor.matmul(
                        pso[:, 0:512], lhsT=hTa[:, t * 128:(t + 1) * 128],
                        rhs=lwt_sb[:, 0:512], start=True, stop=True)
                    nc.tensor.matmul(
                        pso[:, 512:D], lhsT=hTa[:, t * 128:(t + 1) * 128],
                        rhs=lwt_sb[:, 512:D], start=True, stop=True)
                    o_sb = osbp.tile([128, D], BF16, tag="osb")
                    copy_alt(o_sb[:], pso[:])
                    # rows stored in (sl, m, j) order; host unpermutes
                    nc.sync.dma_start(
                        out=out[t * 128:(t + 1) * 128, :], in_=o_sb[:])
                ps_stack.close()

    nc.compile()
    return nc


def _prep_inputs(x, task, fc1_w, fc1_b, route_weights, larger_w):
    R = int(task) + 1
    fw = np.ascontiguousarray(
        fc1_w.reshape(NCOL, D).T.reshape(6, 128, NCOL).transpose(1, 0, 2)
    ).reshape(128, 6 * NCOL).astype(np.float32)
    fb = np.ascontiguousarray(fc1_b.reshape(NCOL, 1)).astype(np.float32)
    lwt = np.ascontiguousarray(larger_w.T).astype(ml_dtypes.bfloat16)
    ident = np.eye(128, dtype=np.float32)
    in_maps = []
    for i in range(N_CORES):
        xt_i = np.ascontiguousarray(
            x[:, i * SL:(i + 1) * SL, :].transpose(2, 1, 0)
        ).reshape(D, SL * B).astype(np.float32)
        # rw_i[t, p, r*256+l] = route_weights[i, r, t*128+p, l]
        rw_i = np.ascontiguousarray(
            route_weights[i, :R].reshape(R, KT, 128, L).transpose(1, 2, 0, 3)
        ).reshape(KT, 128, R * L).astype(np.float32)
        in_maps.append({"xt": xt_i, "fw": fw, "fb": fb, "rw": rw_i,
                        "lwt": lwt, "ident": ident})
    return in_maps


def kernel(x, task, fc1_w, fc1_b, route_weights, larger_w, larger_b,
           _return_results=False):
    x = np.asarray(x, dtype=np.float32)
    fc1_w = np.asarray(fc1_w, dtype=np.float32)
    fc1_b = np.asarray(fc1_b, dtype=np.float32)
    route_weights = np.asarray(route_weights, dtype=np.float32)
    larger_w = np.asarray(larger_w, dtype=np.float32)
    larger_b = np.asarray(larger_b, dtype=np.float32)
    R = int(task) + 1

    if R not in _cache:
        _cache[R] = _build(R)
    nc = _cache[R]

    in_maps = _prep_inputs(x, task, fc1_w, fc1_b, route_weights, larger_w)
    res = bass_utils.run_bass_kernel_spmd(nc, in_maps, list(range(N_CORES)))

    full = _gather(res, larger_b)
    if _return_results:
        return full, res
    return full


def _gather(res, larger_b):
    full = np.empty((B, S, D), dtype=np.float32)
    for i in range(N_CORES):
        # device rows are (sl, m, j); reference order is (m, j*32+sl)
        a = np.asarray(res.results[i]["out"], dtype=np.float32)
        a = a.reshape(SL, BLOC, 8, D).transpose(1, 2, 0, 3).reshape(BLOC, S, D)
        full[i * BLOC:(i + 1) * BLOC] = a
    larger_b = np.asarray(larger_b, dtype=np.float32)
    if np.any(larger_b):
        full = full + larger_b[None, None, :]
    return full
